# revision 10
# baseline (speedup 1.0000x reference)
"""2-layer GCN (GCNConv -> ReLU -> GCNConv -> ReLU) on 8 Trainium2 NeuronCores.

Math: out = relu(A_hat @ relu(A_hat @ X @ W1 + b1) @ W2 + b2),
A_hat = D^-1/2 (A + I) D^-1/2.  Associativity lets us aggregate in the
input feature space of each layer: A_hat @ (X W) == (A_hat @ X) W, so every
aggregated message is 128 features wide.

Distribution: destination nodes (and their in-edges) are sharded across the
8 cores.  Per dst block of 128 rows, the segment-sum is a PE matmul chain:
aggT[f,d] += M_chunk[e,f] (contract e) S_chunk[e,d], where S is a
host-precomputed 0/1 selection matrix (one column per dst slot) and M is the
chunk's 128 source-row messages.  Self-loop terms are added with one
identity matmul from the core's own table block.  dinv scaling is folded
into the tables (dinv[src], on write) and into the relu's per-partition
scalar multiply (dinv[dst], on PSUM readout).

Layer 1 messages (x[src]) are materialized host-side in SBUF layout and
streamed sequentially — no device-side gather at all.  Layer 2 messages
(h1[src]) only exist on device: each core writes its h1 slice, an AllGather
publishes the full table, and batched dma_gather pulls the per-edge rows
(the int16 index limit is dodged by splitting chunks into two streams
gathering from two base-offset views of the table).
"""

import os
import sys

for _p in ("/opt/trn_rl_repo", "/root/.axon_site/_ro/trn_rl_repo"):
    if os.path.isdir(_p) and _p not in sys.path:
        sys.path.append(_p)

import numpy as np
import ml_dtypes

import concourse.bacc as bacc
import concourse.tile as tile
from concourse import mybir
from concourse.bass_utils import run_bass_kernel_spmd

NC = 8          # cores
BLK = 128       # dst rows per aggregation block
CHUNK = 128     # edges per matmul chunk (PE contraction dim)
G = 64          # chunks per dma_gather call (layer 2)
GB = 16         # chunks per msg/S stream DMA
SPLIT = 32768   # int16 index limit for dma_gather


def _pack_idx(flat: np.ndarray) -> np.ndarray:
    """dma_gather index layout: [128, n/16] int16, idx i at [i%16, i//16],
    replicated across the 8 gpsimd cores (partition groups of 16)."""
    n = flat.shape[0]
    assert n % 16 == 0
    return np.ascontiguousarray(np.tile(flat.reshape(n // 16, 16).T, (8, 1)))


class _Plan:
    """Host-side schedule + per-core streams (shared by both layers)."""

    def __init__(self, n_nodes: int, edge_index: np.ndarray):
        assert n_nodes % NC == 0
        self.N = n_nodes
        self.ROWS = n_nodes // NC
        self.NBLK = (self.ROWS + BLK - 1) // BLK
        self.last_rows = self.ROWS - (self.NBLK - 1) * BLK

        src = np.asarray(edge_index[0], dtype=np.int64)
        dst = np.asarray(edge_index[1], dtype=np.int64)

        # self-loops are NOT materialized as edge slots: each block adds its
        # own table rows via an identity matmul.  deg still counts them.
        deg = (np.bincount(dst, minlength=n_nodes) + 1).astype(np.float64)
        self.dinv = (1.0 / np.sqrt(deg)).astype(np.float32)

        core = dst // self.ROWS
        rem = dst % self.ROWS
        blk = rem // BLK
        drel = rem % BLK
        hi = (src >= SPLIT).astype(np.int64)

        key = (core * self.NBLK + blk) * 2 + hi
        counts = np.bincount(key, minlength=NC * self.NBLK * 2).reshape(
            NC, self.NBLK, 2
        )
        K = -(-counts // CHUNK)
        K = K.max(axis=0)  # [NBLK, 2] chunk counts, shared by all cores
        self.Klo = K[:, 0].astype(np.int64)
        self.Khi = K[:, 1].astype(np.int64)
        self.C_lo = int(self.Klo.sum())
        self.C_hi = int(self.Khi.sum())
        self.C = self.C_lo + self.C_hi

        base = lambda k: np.concatenate([[0], np.cumsum(k)])
        self.base_lo = base(self.Klo)
        self.base_hi = base(self.Khi)
        self.base_g = base(self.Klo + self.Khi)

        # per-core streams
        self.S = []        # [128, C*128] bf16 selection columns
        self.slotsrc = []  # [C*128] int64 source node per slot (pad: 0/SPLIT)
        self.idxlo = []    # [128, C_lo*8] int16
        self.idxhi = []    # [128, C_hi*8] int16
        self.dinv_col = []   # [128, NBLK] f32
        self.dinv2_col = []  # [128, NBLK] f32
        for c in range(NC):
            m = core == c
            sa, bb, dd, ss = src[m], blk[m], drel[m], hi[m]
            k2 = bb * 2 + ss
            order = np.argsort(k2, kind="stable")
            sa, bb, dd, ss, k2 = (
                sa[order], bb[order], dd[order], ss[order], k2[order],
            )
            change = np.r_[True, k2[1:] != k2[:-1]]
            startidx = np.flatnonzero(change)
            sizes = np.diff(np.r_[startidx, len(k2)])
            pos = np.arange(len(k2)) - np.repeat(startidx, sizes)
            ch = pos // CHUNK
            lane = pos % CHUNK
            q = self.base_g[bb] + np.where(ss == 0, ch, self.Klo[bb] + ch)
            pstream = np.where(ss == 0, self.base_lo[bb], self.base_hi[bb]) + ch

            S = np.zeros((128, self.C, 128), dtype=ml_dtypes.bfloat16)
            S[lane, q, dd] = 1.0
            self.S.append(np.ascontiguousarray(S.reshape(128, self.C * 128)))

            # slot -> source node (in global chunk order); pads read row 0 of
            # the stream's table view so indices stay in range.
            slotsrc = np.zeros(self.C * CHUNK, np.int64)
            hi_slots = np.zeros(self.C * CHUNK, bool)
            kk = np.repeat(self.Klo, 1)
            for b in range(self.NBLK):
                s0 = (self.base_g[b] + self.Klo[b]) * CHUNK
                s1 = self.base_g[b + 1] * CHUNK
                hi_slots[s0:s1] = True
            slotsrc[hi_slots] = SPLIT
            slot = q * CHUNK + lane
            slotsrc[slot] = sa
            self.slotsrc.append(slotsrc)

            # L2 gather index streams (lo chunks then hi chunks, per stream)
            idx_lo = np.zeros(max(self.C_lo, 1) * CHUNK, np.int16)
            idx_hi = np.zeros(max(self.C_hi, 1) * CHUNK, np.int16)
            lo_m = ss == 0
            idx_lo[pstream[lo_m] * CHUNK + lane[lo_m]] = sa[lo_m].astype(
                np.int16
            )
            if self.C_hi:
                idx_hi[pstream[~lo_m] * CHUNK + lane[~lo_m]] = (
                    sa[~lo_m] - SPLIT
                ).astype(np.int16)
            self.idxlo.append(_pack_idx(idx_lo))
            self.idxhi.append(_pack_idx(idx_hi))

            dv = np.ones(self.NBLK * BLK, np.float32)
            dv[: self.ROWS] = self.dinv[c * self.ROWS : (c + 1) * self.ROWS]
            dv = dv.reshape(self.NBLK, BLK).T.copy()  # [128, NBLK]
            self.dinv_col.append(dv)
            self.dinv2_col.append(dv * dv)

    def msg1(self, xg_full: np.ndarray, c: int) -> np.ndarray:
        """Layer-1 message stream for core c, already in SBUF layout
        [128 slot-lane partitions, C*128 (chunk-major, feature-minor)]."""
        m = xg_full[self.slotsrc[c]]  # [C*128, F]
        F = m.shape[1]
        return np.ascontiguousarray(
            m.reshape(self.C, CHUNK, F).transpose(1, 0, 2).reshape(
                128, self.C * F
            )
        )

    def signature(self, f_in, f_out, has_b1, has_b2):
        return (
            self.N, f_in, f_out, has_b1, has_b2,
            tuple(self.Klo), tuple(self.Khi),
        )


def _build(plan: _Plan, f_in: int, f_out: int, has_b1: bool, has_b2: bool):
    """Build + compile the SPMD Bass program (one NEFF, runs on all 8 cores)."""
    N, ROWS, NBLK = plan.N, plan.ROWS, plan.NBLK
    C, C_lo, C_hi = plan.C, plan.C_lo, plan.C_hi
    bf16, f32, i16 = mybir.dt.bfloat16, mybir.dt.float32, mybir.dt.int16

    nc = bacc.Bacc("TRN2", target_bir_lowering=False, debug=False,
                   enable_asserts=True, num_devices=NC)

    xgl = nc.dram_tensor("xgl", [ROWS, f_in], bf16, kind="ExternalInput")
    msg1 = nc.dram_tensor("msg1", [128, C * f_in], bf16, kind="ExternalInput")
    smat = nc.dram_tensor("smat", [128, C * 128], bf16, kind="ExternalInput")
    ident_in = nc.dram_tensor("ident", [128, 128], bf16, kind="ExternalInput")
    w1 = nc.dram_tensor("w1", [f_in, f_in], f32, kind="ExternalInput")
    w2 = nc.dram_tensor("w2", [f_in, f_out], f32, kind="ExternalInput")
    if has_b1:
        b1bc = nc.dram_tensor("b1bc", [128, f_in], f32, kind="ExternalInput")
    if has_b2:
        b2bc = nc.dram_tensor("b2bc", [128, f_out], f32, kind="ExternalInput")
    idxlo = nc.dram_tensor("idxlo", [128, max(C_lo, 1) * 8], i16,
                           kind="ExternalInput")
    if C_hi:
        idxhi = nc.dram_tensor("idxhi", [128, C_hi * 8], i16,
                               kind="ExternalInput")
    dinv_c = nc.dram_tensor("dinv_c", [128, NBLK], f32, kind="ExternalInput")
    dinv2_c = nc.dram_tensor("dinv2_c", [128, NBLK], f32, kind="ExternalInput")
    out_ext = nc.dram_tensor("out", [ROWS, f_out], f32, kind="ExternalOutput")

    with tile.TileContext(nc) as tc:
        with (
            tc.tile_pool(name="meta", bufs=1) as pm,
            tc.tile_pool(name="work", bufs=2) as pw,
            tc.tile_pool(name="psum", space="PSUM", bufs=2) as pp,
            tc.tile_pool(name="dram", space="DRAM", bufs=1) as pd,
        ):
            # ---- persistent metadata in SBUF ----
            ident_t = pm.tile([128, 128], bf16)
            nc.sync.dma_start(ident_t[:], ident_in[:])
            idxlo_t = pm.tile([128, max(C_lo, 1) * 8], i16)
            nc.sync.dma_start(idxlo_t[:], idxlo[:])
            if C_hi:
                idxhi_t = pm.tile([128, C_hi * 8], i16)
                nc.sync.dma_start(idxhi_t[:], idxhi[:])
            dinv_t = pm.tile([128, NBLK], f32)
            nc.sync.dma_start(dinv_t[:], dinv_c[:])
            dinv2_t = pm.tile([128, NBLK], f32)
            nc.sync.dma_start(dinv2_t[:], dinv2_c[:])

            w1f = pm.tile([f_in, f_in], f32)
            nc.sync.dma_start(w1f[:], w1[:])
            w1_t = pm.tile([f_in, f_in], bf16)
            nc.vector.tensor_copy(w1_t[:], w1f[:])
            w2f = pm.tile([f_in, f_out], f32)
            nc.sync.dma_start(w2f[:], w2[:])
            w2_t = pm.tile([f_in, f_out], bf16)
            nc.vector.tensor_copy(w2_t[:], w2f[:])
            if has_b1:
                b1_t = pm.tile([128, f_in], f32)
                nc.sync.dma_start(b1_t[:], b1bc[:])
            if has_b2:
                b2_t = pm.tile([128, f_out], f32)
                nc.sync.dma_start(b2_t[:], b2bc[:])

            # The DMAGatherAnt instruction struct only has room for ONE sync
            # wait command (walrus setupSyncWait limit).  Absorb the idx-load
            # dependencies into a throwaway gpsimd DMA so every dma_gather
            # needs at most one wait (collective done / msg-buf WAR).
            scratch = pm.tile([1, 16], i16, name="scratch")
            nc.gpsimd.dma_start(scratch[0:1, 0:16], idxlo_t[0:1, 0:16])
            if C_hi:
                nc.gpsimd.dma_start(scratch[0:1, 0:16], idxhi_t[0:1, 0:16])

            ag2_in = pd.tile([ROWS, f_in], bf16)
            h1_tab = pd.tile([N, f_in], bf16, addr_space="Shared")

            def stream_tile(dram_src, q0, n, tag):
                """[128, n*128] slice of a host-precomputed [128, C*128]
                stream, loaded contiguously at line rate."""
                t = pw.tile([128, GB * 128], bf16, tag=tag, bufs=6,
                            name=f"{tag}_{q0}")
                nc.sync.dma_start(
                    t[:, : n * 128], dram_src[:, q0 * 128 : (q0 + n) * 128]
                )
                return t

            def emit_layer(layer, w_t, fo, bias_t, scale_t, local_tab):
                # msg source: layer 0 streams msg1; layer 1 gathers h1_tab
                msg_tiles = {}
                s_tiles = {}

                def get_msg(q):
                    if layer == 0:
                        j, col = divmod(q, GB)
                        if j not in msg_tiles:
                            n = min(GB, C - j * GB)
                            msg_tiles[j] = stream_tile(msg1, j * GB, n, "msg")
                        return msg_tiles[j], col
                    # layer 1: chunk q -> (stream, pos)
                    b = int(np.searchsorted(plan.base_g, q, side="right")) - 1
                    i = q - int(plan.base_g[b])
                    if i < int(plan.Klo[b]):
                        s, p = 0, int(plan.base_lo[b]) + i
                    else:
                        s, p = 1, int(plan.base_hi[b]) + i - int(plan.Klo[b])
                    j, col = divmod(p, G)
                    if (s, j) not in msg_tiles:
                        cs = C_lo if s == 0 else C_hi
                        n = min(G, cs - j * G)
                        mt = pw.tile([128, G * CHUNK], bf16, tag="gat",
                                     bufs=4, name=f"gat_{s}_{j}")
                        view = (h1_tab[0 : min(SPLIT, N), :] if s == 0
                                else h1_tab[SPLIT:N, :])
                        it = idxlo_t if s == 0 else idxhi_t
                        nc.gpsimd.dma_gather(
                            mt[:, : n * CHUNK].rearrange(
                                "p (g e) -> p g e", e=f_in),
                            view,
                            it[:, j * G * 8 : (j * G + n) * 8],
                            n * CHUNK, n * CHUNK, f_in,
                            single_packet=False,
                        )
                        msg_tiles[(s, j)] = mt
                    return msg_tiles[(s, j)], col

                def get_s(q):
                    j, col = divmod(q, GB)
                    if (layer, j) not in s_tiles:
                        n = min(GB, C - j * GB)
                        s_tiles[(layer, j)] = stream_tile(
                            smat, j * GB, n, "sel")
                    return s_tiles[(layer, j)], col

                q = 0
                for b in range(NBLK):
                    rows = plan.last_rows if b == NBLK - 1 else BLK
                    nch = int(plan.Klo[b]) + int(plan.Khi[b])
                    psum = pp.tile([128, 128], f32, tag="agg", bufs=4,
                                   name=f"agg_l{layer}_{b}")
                    # self-loop contribution: psum[f,d] += local[dd,f]*I[dd,d]
                    loc = pw.tile([128, 128], bf16, tag="loc", bufs=4,
                                  name=f"loc_l{layer}_{b}")
                    nc.sync.dma_start(
                        loc[:rows, :],
                        local_tab[b * BLK : b * BLK + rows, :],
                    )
                    # rows < BLK leaves stale data in loc[rows:]; identity
                    # routing sends row dd only to psum column dd, and
                    # columns >= rows are never read back, so it's harmless.
                    nc.tensor.matmul(psum[:], lhsT=loc[:], rhs=ident_t[:],
                                     start=True, stop=(nch == 0))
                    for i in range(nch):
                        mt, mcol = get_msg(q)
                        st, scol = get_s(q)
                        nc.tensor.matmul(
                            psum[:],
                            lhsT=mt[:, mcol * CHUNK : (mcol + 1) * CHUNK],
                            rhs=st[:, scol * 128 : (scol + 1) * 128],
                            start=False, stop=(i == nch - 1),
                        )
                        q += 1
                    aggT = pw.tile([128, 128], bf16, tag="aggT", bufs=4,
                                   name=f"aggT_l{layer}_{b}")
                    nc.vector.tensor_copy(aggT[:], psum[:])
                    ph = pp.tile([128, fo], f32, tag="hout", bufs=2,
                                 name=f"ph_l{layer}_{b}")
                    nc.tensor.matmul(ph[:], lhsT=aggT[:], rhs=w_t[:],
                                     start=True, stop=True)
                    hsb = pw.tile([128, fo], bf16 if layer == 0 else f32,
                                  tag=f"hsb{layer}", bufs=4,
                                  name=f"hsb_l{layer}_{b}")
                    if bias_t is None:
                        # relu(dinv*x)*k == max(x,0)*(dinv*k): one DVE op
                        nc.vector.tensor_scalar(
                            out=hsb[:], in0=ph[:],
                            scalar1=0.0,
                            scalar2=scale_t[:, b : b + 1],
                            op0=mybir.AluOpType.max,
                            op1=mybir.AluOpType.mult,
                        )
                    else:
                        tmp = pw.tile([128, fo], f32, tag=f"tmp{layer}",
                                      bufs=2, name=f"tmp_l{layer}_{b}")
                        nc.vector.scalar_tensor_tensor(
                            out=tmp[:], in0=ph[:],
                            scalar=dinv_t[:, b : b + 1], in1=bias_t[:],
                            op0=mybir.AluOpType.mult,
                            op1=mybir.AluOpType.add,
                        )
                        if layer == 0:
                            nc.vector.tensor_scalar(
                                out=hsb[:], in0=tmp[:], scalar1=0.0,
                                scalar2=dinv_t[:, b : b + 1],
                                op0=mybir.AluOpType.max,
                                op1=mybir.AluOpType.mult,
                            )
                        else:
                            nc.vector.tensor_scalar(
                                out=hsb[:], in0=tmp[:], scalar1=0.0,
                                scalar2=None,
                                op0=mybir.AluOpType.max,
                                op1=mybir.AluOpType.bypass,
                            )
                    dst_rows = slice(b * BLK, b * BLK + rows)
                    if layer == 0:
                        nc.sync.dma_start(ag2_in[dst_rows, :], hsb[:rows, :])
                    else:
                        nc.sync.dma_start(out_ext[dst_rows, :], hsb[:rows, :])

            # layer 1: table rows pre-scaled by dinv[src] on host; output row
            # d is relu(dinv_d * (agg @ W1) + b1) * dinv_d (layer-2 table)
            emit_layer(0, w1_t, f_in, b1_t if has_b1 else None,
                       dinv2_t, xgl)
            nc.gpsimd.collective_compute(
                "AllGather", mybir.AluOpType.bypass,
                replica_groups=[list(range(NC))],
                ins=[ag2_in[:]], outs=[h1_tab[:]],
            )
            # layer 2: out row d = relu(dinv_d * (agg @ W2) + b2)
            emit_layer(1, w2_t, f_out, b2_t if has_b2 else None,
                       dinv_t, ag2_in)

    nc.compile()
    return nc


_cache: dict = {}


def _get_program(plan, f_in, f_out, has_b1, has_b2):
    key = plan.signature(f_in, f_out, has_b1, has_b2)
    if key not in _cache:
        _cache[key] = _build(plan, f_in, f_out, has_b1, has_b2)
    return _cache[key]


def _run(x, edge_index, W1, b1, W2, b2, trace=False, trace_cores=None):
    x = np.asarray(x, dtype=np.float32)
    W1 = np.asarray(W1, dtype=np.float32)
    W2 = np.asarray(W2, dtype=np.float32)
    b1 = np.asarray(b1, dtype=np.float32)
    b2 = np.asarray(b2, dtype=np.float32)
    N, f_in = x.shape
    f_out = W2.shape[1]
    has_b1 = bool(np.any(b1))
    has_b2 = bool(np.any(b2))

    plan = _Plan(N, np.asarray(edge_index))
    nc = _get_program(plan, f_in, f_out, has_b1, has_b2)

    in_maps = _make_in_maps(plan, x, W1, b1, W2, b2)

    kw = {}
    if trace:
        kw = dict(trace=True)
        if trace_cores is not None:
            kw["trace_cores"] = trace_cores
    res = run_bass_kernel_spmd(nc, in_maps, core_ids=list(range(NC)), **kw)
    out = np.concatenate([res.results[c]["out"] for c in range(NC)], axis=0)
    return out, res


def _make_in_maps(plan, x, W1, b1, W2, b2):
    has_b1 = bool(np.any(b1))
    has_b2 = bool(np.any(b2))
    xg_full = (x * plan.dinv[:, None]).astype(ml_dtypes.bfloat16)
    ident = np.eye(128, dtype=ml_dtypes.bfloat16)
    in_maps = []
    for c in range(NC):
        m = dict(
            xgl=np.ascontiguousarray(
                xg_full[c * plan.ROWS : (c + 1) * plan.ROWS]
            ),
            msg1=plan.msg1(xg_full, c),
            smat=plan.S[c],
            ident=ident,
            w1=W1, w2=W2,
            idxlo=plan.idxlo[c],
            dinv_c=plan.dinv_col[c],
            dinv2_c=plan.dinv2_col[c],
        )
        if plan.C_hi:
            m["idxhi"] = plan.idxhi[c]
        if has_b1:
            m["b1bc"] = np.ascontiguousarray(np.tile(b1, (128, 1)))
        if has_b2:
            m["b2bc"] = np.ascontiguousarray(np.tile(b2, (128, 1)))
        in_maps.append(m)
    return in_maps


def kernel(x, edge_index, W1, b1, W2, b2):
    out, _ = _run(x, edge_index, W1, b1, W2, b2)
    return out


# revision 11
# speedup vs baseline: 1.1072x; 1.1072x over previous
"""2-layer GCN (GCNConv -> ReLU -> GCNConv -> ReLU) on 8 Trainium2 NeuronCores.

Math: out = relu(A_hat @ relu(A_hat @ X @ W1 + b1) @ W2 + b2),
A_hat = D^-1/2 (A + I) D^-1/2.  Associativity lets us aggregate in the
input feature space of each layer: A_hat @ (X W) == (A_hat @ X) W, so every
aggregated message is 128 features wide.

Distribution: destination nodes (and their in-edges) are sharded across the
8 cores.  Per dst block of 128 rows, the segment-sum is a PE matmul chain:
aggT[f,d] += M_chunk[e,f] (contract e) S_chunk[e,d], where S is a
host-precomputed 0/1 selection matrix (one column per dst slot) and M is the
chunk's 128 source-row messages.  Self-loop terms are added with one
identity matmul from the core's own table block.  dinv scaling is folded
into the tables (dinv[src], on write) and into the relu's per-partition
scalar multiply (dinv[dst], on PSUM readout).

Layer 1 messages (x[src]) are materialized host-side in SBUF layout and
streamed sequentially — no device-side gather at all.  Layer 2 messages
(h1[src]) only exist on device: each core writes its h1 slice, an AllGather
publishes the full table, and batched dma_gather pulls the per-edge rows
(the int16 index limit is dodged by splitting chunks into two streams
gathering from two base-offset views of the table).
"""

import os
import sys

for _p in ("/opt/trn_rl_repo", "/root/.axon_site/_ro/trn_rl_repo"):
    if os.path.isdir(_p) and _p not in sys.path:
        sys.path.append(_p)

import numpy as np
import ml_dtypes

import concourse.bacc as bacc
import concourse.tile as tile
from concourse import mybir
from concourse.bass_utils import run_bass_kernel_spmd

NC = 8          # cores
BLK = 128       # dst rows per aggregation block
CHUNK = 128     # edges per matmul chunk (PE contraction dim)
G = 64          # chunks per dma_gather call (layer 2)
GB = 16         # chunks per msg/S stream DMA
SPLIT = 32768   # int16 index limit for dma_gather


def _pack_idx(flat: np.ndarray) -> np.ndarray:
    """dma_gather index layout: [128, n/16] int16, idx i at [i%16, i//16],
    replicated across the 8 gpsimd cores (partition groups of 16)."""
    n = flat.shape[0]
    assert n % 16 == 0
    return np.ascontiguousarray(np.tile(flat.reshape(n // 16, 16).T, (8, 1)))


class _Plan:
    """Host-side schedule + per-core streams (shared by both layers)."""

    def __init__(self, n_nodes: int, edge_index: np.ndarray):
        assert n_nodes % NC == 0
        self.N = n_nodes
        self.ROWS = n_nodes // NC
        self.NBLK = (self.ROWS + BLK - 1) // BLK
        self.last_rows = self.ROWS - (self.NBLK - 1) * BLK

        src = np.asarray(edge_index[0], dtype=np.int64)
        dst = np.asarray(edge_index[1], dtype=np.int64)

        # self-loops are NOT materialized as edge slots: each block adds its
        # own table rows via an identity matmul.  deg still counts them.
        deg = (np.bincount(dst, minlength=n_nodes) + 1).astype(np.float64)
        self.dinv = (1.0 / np.sqrt(deg)).astype(np.float32)

        core = dst // self.ROWS
        rem = dst % self.ROWS
        blk = rem // BLK
        drel = rem % BLK
        hi = (src >= SPLIT).astype(np.int64)

        key = (core * self.NBLK + blk) * 2 + hi
        counts = np.bincount(key, minlength=NC * self.NBLK * 2).reshape(
            NC, self.NBLK, 2
        )
        K = -(-counts // CHUNK)
        K = K.max(axis=0)  # [NBLK, 2] chunk counts, shared by all cores
        self.Klo = K[:, 0].astype(np.int64)
        self.Khi = K[:, 1].astype(np.int64)
        self.C_lo = int(self.Klo.sum())
        self.C_hi = int(self.Khi.sum())
        self.C = self.C_lo + self.C_hi

        base = lambda k: np.concatenate([[0], np.cumsum(k)])
        self.base_lo = base(self.Klo)
        self.base_hi = base(self.Khi)
        self.base_g = base(self.Klo + self.Khi)

        # per-core streams
        self.S = []        # [128, C*128] bf16 selection columns
        self.slotsrc = []  # [C*128] int64 source node per slot (pad: 0/SPLIT)
        self.idxlo = []    # [128, C_lo*8] int16
        self.idxhi = []    # [128, C_hi*8] int16
        self.dinv_col = []   # [128, NBLK] f32
        self.dinv2_col = []  # [128, NBLK] f32
        for c in range(NC):
            m = core == c
            sa, bb, dd, ss = src[m], blk[m], drel[m], hi[m]
            k2 = bb * 2 + ss
            order = np.argsort(k2, kind="stable")
            sa, bb, dd, ss, k2 = (
                sa[order], bb[order], dd[order], ss[order], k2[order],
            )
            change = np.r_[True, k2[1:] != k2[:-1]]
            startidx = np.flatnonzero(change)
            sizes = np.diff(np.r_[startidx, len(k2)])
            pos = np.arange(len(k2)) - np.repeat(startidx, sizes)
            ch = pos // CHUNK
            lane = pos % CHUNK
            q = self.base_g[bb] + np.where(ss == 0, ch, self.Klo[bb] + ch)
            pstream = np.where(ss == 0, self.base_lo[bb], self.base_hi[bb]) + ch

            S = np.zeros((128, self.C, 128), dtype=ml_dtypes.bfloat16)
            S[lane, q, dd] = 1.0
            self.S.append(np.ascontiguousarray(S.reshape(128, self.C * 128)))

            # slot -> source node (in global chunk order); pads read row 0 of
            # the stream's table view so indices stay in range.
            slotsrc = np.zeros(self.C * CHUNK, np.int64)
            hi_slots = np.zeros(self.C * CHUNK, bool)
            kk = np.repeat(self.Klo, 1)
            for b in range(self.NBLK):
                s0 = (self.base_g[b] + self.Klo[b]) * CHUNK
                s1 = self.base_g[b + 1] * CHUNK
                hi_slots[s0:s1] = True
            slotsrc[hi_slots] = SPLIT
            slot = q * CHUNK + lane
            slotsrc[slot] = sa
            self.slotsrc.append(slotsrc)

            # L2 gather index streams (lo chunks then hi chunks, per stream)
            idx_lo = np.zeros(max(self.C_lo, 1) * CHUNK, np.int16)
            idx_hi = np.zeros(max(self.C_hi, 1) * CHUNK, np.int16)
            lo_m = ss == 0
            idx_lo[pstream[lo_m] * CHUNK + lane[lo_m]] = sa[lo_m].astype(
                np.int16
            )
            if self.C_hi:
                idx_hi[pstream[~lo_m] * CHUNK + lane[~lo_m]] = (
                    sa[~lo_m] - SPLIT
                ).astype(np.int16)
            self.idxlo.append(_pack_idx(idx_lo))
            self.idxhi.append(_pack_idx(idx_hi))

            dv = np.ones(self.NBLK * BLK, np.float32)
            dv[: self.ROWS] = self.dinv[c * self.ROWS : (c + 1) * self.ROWS]
            dv = dv.reshape(self.NBLK, BLK).T.copy()  # [128, NBLK]
            self.dinv_col.append(dv)
            self.dinv2_col.append(dv * dv)

    def msg1(self, xg_full: np.ndarray, c: int) -> np.ndarray:
        """Layer-1 message stream for core c, already in SBUF layout
        [128 slot-lane partitions, C*128 (chunk-major, feature-minor)]."""
        m = xg_full[self.slotsrc[c]]  # [C*128, F]
        F = m.shape[1]
        return np.ascontiguousarray(
            m.reshape(self.C, CHUNK, F).transpose(1, 0, 2).reshape(
                128, self.C * F
            )
        )

    def signature(self, f_in, f_out, has_b1, has_b2):
        return (
            self.N, f_in, f_out, has_b1, has_b2,
            tuple(self.Klo), tuple(self.Khi),
        )


def _build(plan: _Plan, f_in: int, f_out: int, has_b1: bool, has_b2: bool):
    """Build + compile the SPMD Bass program (one NEFF, runs on all 8 cores)."""
    N, ROWS, NBLK = plan.N, plan.ROWS, plan.NBLK
    C, C_lo, C_hi = plan.C, plan.C_lo, plan.C_hi
    bf16, f32, i16 = mybir.dt.bfloat16, mybir.dt.float32, mybir.dt.int16

    nc = bacc.Bacc("TRN2", target_bir_lowering=False, debug=False,
                   enable_asserts=True, num_devices=NC)

    xgl = nc.dram_tensor("xgl", [ROWS, f_in], bf16, kind="ExternalInput")
    msg1 = nc.dram_tensor("msg1", [128, C * f_in], bf16, kind="ExternalInput")
    smat = nc.dram_tensor("smat", [128, C * 128], bf16, kind="ExternalInput")
    ident_in = nc.dram_tensor("ident", [128, 128], bf16, kind="ExternalInput")
    w1 = nc.dram_tensor("w1", [f_in, f_in], f32, kind="ExternalInput")
    w2 = nc.dram_tensor("w2", [f_in, f_out], f32, kind="ExternalInput")
    if has_b1:
        b1bc = nc.dram_tensor("b1bc", [128, f_in], f32, kind="ExternalInput")
    if has_b2:
        b2bc = nc.dram_tensor("b2bc", [128, f_out], f32, kind="ExternalInput")
    idxlo = nc.dram_tensor("idxlo", [128, max(C_lo, 1) * 8], i16,
                           kind="ExternalInput")
    if C_hi:
        idxhi = nc.dram_tensor("idxhi", [128, C_hi * 8], i16,
                               kind="ExternalInput")
    dinv_c = nc.dram_tensor("dinv_c", [128, NBLK], f32, kind="ExternalInput")
    dinv2_c = nc.dram_tensor("dinv2_c", [128, NBLK], f32, kind="ExternalInput")
    out_ext = nc.dram_tensor("out", [ROWS, f_out], f32, kind="ExternalOutput")

    with tile.TileContext(nc) as tc:
        with (
            tc.tile_pool(name="meta", bufs=1) as pm,
            tc.tile_pool(name="work", bufs=2) as pw,
            tc.tile_pool(name="psum", space="PSUM", bufs=2) as pp,
            tc.tile_pool(name="dram", space="DRAM", bufs=1) as pd,
        ):
            # ---- persistent metadata in SBUF ----
            ident_t = pm.tile([128, 128], bf16)
            nc.sync.dma_start(ident_t[:], ident_in[:])
            idxlo_t = pm.tile([128, max(C_lo, 1) * 8], i16)
            nc.sync.dma_start(idxlo_t[:], idxlo[:])
            if C_hi:
                idxhi_t = pm.tile([128, C_hi * 8], i16)
                nc.sync.dma_start(idxhi_t[:], idxhi[:])
            dinv_t = pm.tile([128, NBLK], f32)
            nc.sync.dma_start(dinv_t[:], dinv_c[:])
            dinv2_t = pm.tile([128, NBLK], f32)
            nc.sync.dma_start(dinv2_t[:], dinv2_c[:])

            w1f = pm.tile([f_in, f_in], f32)
            nc.sync.dma_start(w1f[:], w1[:])
            w1_t = pm.tile([f_in, f_in], bf16)
            nc.vector.tensor_copy(w1_t[:], w1f[:])
            w2f = pm.tile([f_in, f_out], f32)
            nc.sync.dma_start(w2f[:], w2[:])
            w2_t = pm.tile([f_in, f_out], bf16)
            nc.vector.tensor_copy(w2_t[:], w2f[:])
            if has_b1:
                b1_t = pm.tile([128, f_in], f32)
                nc.sync.dma_start(b1_t[:], b1bc[:])
            if has_b2:
                b2_t = pm.tile([128, f_out], f32)
                nc.sync.dma_start(b2_t[:], b2bc[:])

            # The DMAGatherAnt instruction struct only has room for ONE sync
            # wait command (walrus setupSyncWait limit).  Absorb the idx-load
            # dependencies into a throwaway gpsimd DMA so every dma_gather
            # needs at most one wait (collective done / msg-buf WAR).
            scratch = pm.tile([1, 16], i16, name="scratch")
            nc.gpsimd.dma_start(scratch[0:1, 0:16], idxlo_t[0:1, 0:16])
            if C_hi:
                nc.gpsimd.dma_start(scratch[0:1, 0:16], idxhi_t[0:1, 0:16])

            ag2_in = pd.tile([ROWS, f_in], bf16)
            h1_tab = pd.tile([N, f_in], bf16, addr_space="Shared")

            def stream_tile(dram_src, q0, n, tag):
                """[128, n*128] slice of a host-precomputed [128, C*128]
                stream, loaded contiguously at line rate."""
                t = pw.tile([128, GB * 128], bf16, tag=tag, bufs=4,
                            name=f"{tag}_{q0}")
                nc.sync.dma_start(
                    t[:, : n * 128], dram_src[:, q0 * 128 : (q0 + n) * 128]
                )
                return t

            def emit_layer(layer, w_t, fo, bias_t, scale_t, local_tab):
                # msg source: layer 0 streams msg1; layer 1 gathers h1_tab
                msg_tiles = {}
                s_tiles = {}

                def get_msg(q):
                    if layer == 0:
                        j, col = divmod(q, GB)
                        if j not in msg_tiles:
                            n = min(GB, C - j * GB)
                            msg_tiles[j] = stream_tile(msg1, j * GB, n, "msg")
                        return msg_tiles[j], col
                    # layer 1: chunk q -> (stream, pos)
                    b = int(np.searchsorted(plan.base_g, q, side="right")) - 1
                    i = q - int(plan.base_g[b])
                    if i < int(plan.Klo[b]):
                        s, p = 0, int(plan.base_lo[b]) + i
                    else:
                        s, p = 1, int(plan.base_hi[b]) + i - int(plan.Klo[b])
                    j, col = divmod(p, G)
                    if (s, j) not in msg_tiles:
                        cs = C_lo if s == 0 else C_hi
                        n = min(G, cs - j * G)
                        mt = pw.tile([128, G * CHUNK], bf16, tag="gat",
                                     bufs=6, name=f"gat_{s}_{j}")
                        view = (h1_tab[0 : min(SPLIT, N), :] if s == 0
                                else h1_tab[SPLIT:N, :])
                        it = idxlo_t if s == 0 else idxhi_t
                        nc.gpsimd.dma_gather(
                            mt[:, : n * CHUNK].rearrange(
                                "p (g e) -> p g e", e=f_in),
                            view,
                            it[:, j * G * 8 : (j * G + n) * 8],
                            n * CHUNK, n * CHUNK, f_in,
                            single_packet=False,
                        )
                        msg_tiles[(s, j)] = mt
                    return msg_tiles[(s, j)], col

                def get_s(q):
                    j, col = divmod(q, GB)
                    if (layer, j) not in s_tiles:
                        n = min(GB, C - j * GB)
                        s_tiles[(layer, j)] = stream_tile(
                            smat, j * GB, n, "sel")
                    return s_tiles[(layer, j)], col

                q = 0
                for b in range(NBLK):
                    rows = plan.last_rows if b == NBLK - 1 else BLK
                    nch = int(plan.Klo[b]) + int(plan.Khi[b])
                    psum = pp.tile([128, 128], f32, tag="agg", bufs=4,
                                   name=f"agg_l{layer}_{b}")
                    # self-loop contribution: psum[f,d] += local[dd,f]*I[dd,d]
                    loc = pw.tile([128, 128], bf16, tag="loc", bufs=4,
                                  name=f"loc_l{layer}_{b}")
                    nc.sync.dma_start(
                        loc[:rows, :],
                        local_tab[b * BLK : b * BLK + rows, :],
                    )
                    # rows < BLK leaves stale data in loc[rows:]; identity
                    # routing sends row dd only to psum column dd, and
                    # columns >= rows are never read back, so it's harmless.
                    nc.tensor.matmul(psum[:], lhsT=loc[:], rhs=ident_t[:],
                                     start=True, stop=(nch == 0))
                    for i in range(nch):
                        mt, mcol = get_msg(q)
                        st, scol = get_s(q)
                        nc.tensor.matmul(
                            psum[:],
                            lhsT=mt[:, mcol * CHUNK : (mcol + 1) * CHUNK],
                            rhs=st[:, scol * 128 : (scol + 1) * 128],
                            start=False, stop=(i == nch - 1),
                        )
                        q += 1
                    aggT = pw.tile([128, 128], bf16, tag="aggT", bufs=4,
                                   name=f"aggT_l{layer}_{b}")
                    nc.vector.tensor_copy(aggT[:], psum[:])
                    ph = pp.tile([128, fo], f32, tag="hout", bufs=2,
                                 name=f"ph_l{layer}_{b}")
                    nc.tensor.matmul(ph[:], lhsT=aggT[:], rhs=w_t[:],
                                     start=True, stop=True)
                    hsb = pw.tile([128, fo], bf16 if layer == 0 else f32,
                                  tag=f"hsb{layer}", bufs=4,
                                  name=f"hsb_l{layer}_{b}")
                    if bias_t is None:
                        # relu(dinv*x)*k == max(x,0)*(dinv*k): one DVE op
                        nc.vector.tensor_scalar(
                            out=hsb[:], in0=ph[:],
                            scalar1=0.0,
                            scalar2=scale_t[:, b : b + 1],
                            op0=mybir.AluOpType.max,
                            op1=mybir.AluOpType.mult,
                        )
                    else:
                        tmp = pw.tile([128, fo], f32, tag=f"tmp{layer}",
                                      bufs=2, name=f"tmp_l{layer}_{b}")
                        nc.vector.scalar_tensor_tensor(
                            out=tmp[:], in0=ph[:],
                            scalar=dinv_t[:, b : b + 1], in1=bias_t[:],
                            op0=mybir.AluOpType.mult,
                            op1=mybir.AluOpType.add,
                        )
                        if layer == 0:
                            nc.vector.tensor_scalar(
                                out=hsb[:], in0=tmp[:], scalar1=0.0,
                                scalar2=dinv_t[:, b : b + 1],
                                op0=mybir.AluOpType.max,
                                op1=mybir.AluOpType.mult,
                            )
                        else:
                            nc.vector.tensor_scalar(
                                out=hsb[:], in0=tmp[:], scalar1=0.0,
                                scalar2=None,
                                op0=mybir.AluOpType.max,
                                op1=mybir.AluOpType.bypass,
                            )
                    dst_rows = slice(b * BLK, b * BLK + rows)
                    if layer == 0:
                        nc.sync.dma_start(ag2_in[dst_rows, :], hsb[:rows, :])
                    else:
                        nc.sync.dma_start(out_ext[dst_rows, :], hsb[:rows, :])

            # layer 1: table rows pre-scaled by dinv[src] on host; output row
            # d is relu(dinv_d * (agg @ W1) + b1) * dinv_d (layer-2 table)
            emit_layer(0, w1_t, f_in, b1_t if has_b1 else None,
                       dinv2_t, xgl)
            nc.gpsimd.collective_compute(
                "AllGather", mybir.AluOpType.bypass,
                replica_groups=[list(range(NC))],
                ins=[ag2_in[:]], outs=[h1_tab[:]],
            )
            # layer 2: out row d = relu(dinv_d * (agg @ W2) + b2)
            emit_layer(1, w2_t, f_out, b2_t if has_b2 else None,
                       dinv_t, ag2_in)

    nc.compile()
    return nc


_cache: dict = {}


def _get_program(plan, f_in, f_out, has_b1, has_b2):
    key = plan.signature(f_in, f_out, has_b1, has_b2)
    if key not in _cache:
        _cache[key] = _build(plan, f_in, f_out, has_b1, has_b2)
    return _cache[key]


def _run(x, edge_index, W1, b1, W2, b2, trace=False, trace_cores=None):
    x = np.asarray(x, dtype=np.float32)
    W1 = np.asarray(W1, dtype=np.float32)
    W2 = np.asarray(W2, dtype=np.float32)
    b1 = np.asarray(b1, dtype=np.float32)
    b2 = np.asarray(b2, dtype=np.float32)
    N, f_in = x.shape
    f_out = W2.shape[1]
    has_b1 = bool(np.any(b1))
    has_b2 = bool(np.any(b2))

    plan = _Plan(N, np.asarray(edge_index))
    nc = _get_program(plan, f_in, f_out, has_b1, has_b2)

    in_maps = _make_in_maps(plan, x, W1, b1, W2, b2)

    kw = {}
    if trace:
        kw = dict(trace=True)
        if trace_cores is not None:
            kw["trace_cores"] = trace_cores
    res = run_bass_kernel_spmd(nc, in_maps, core_ids=list(range(NC)), **kw)
    out = np.concatenate([res.results[c]["out"] for c in range(NC)], axis=0)
    return out, res


def _make_in_maps(plan, x, W1, b1, W2, b2):
    has_b1 = bool(np.any(b1))
    has_b2 = bool(np.any(b2))
    xg_full = (x * plan.dinv[:, None]).astype(ml_dtypes.bfloat16)
    ident = np.eye(128, dtype=ml_dtypes.bfloat16)
    in_maps = []
    for c in range(NC):
        m = dict(
            xgl=np.ascontiguousarray(
                xg_full[c * plan.ROWS : (c + 1) * plan.ROWS]
            ),
            msg1=plan.msg1(xg_full, c),
            smat=plan.S[c],
            ident=ident,
            w1=W1, w2=W2,
            idxlo=plan.idxlo[c],
            dinv_c=plan.dinv_col[c],
            dinv2_c=plan.dinv2_col[c],
        )
        if plan.C_hi:
            m["idxhi"] = plan.idxhi[c]
        if has_b1:
            m["b1bc"] = np.ascontiguousarray(np.tile(b1, (128, 1)))
        if has_b2:
            m["b2bc"] = np.ascontiguousarray(np.tile(b2, (128, 1)))
        in_maps.append(m)
    return in_maps


def kernel(x, edge_index, W1, b1, W2, b2):
    out, _ = _run(x, edge_index, W1, b1, W2, b2)
    return out


# revision 12
# speedup vs baseline: 1.2407x; 1.1205x over previous
"""2-layer GCN (GCNConv -> ReLU -> GCNConv -> ReLU) on 8 Trainium2 NeuronCores.

Math: out = relu(A_hat @ relu(A_hat @ X @ W1 + b1) @ W2 + b2),
A_hat = D^-1/2 (A + I) D^-1/2.  Associativity lets us aggregate in the
input feature space of each layer: A_hat @ (X W) == (A_hat @ X) W, so every
aggregated message is 128 features wide.

Distribution: destination nodes (and their in-edges) are sharded across the
8 cores.  Per dst block of 128 rows, the segment-sum is a PE matmul chain:
aggT[f,d] += M_chunk[e,f] (contract e) S_chunk[e,d], where S is a
host-precomputed 0/1 selection matrix (one column per dst slot) and M is the
chunk's 128 source-row messages.  Self-loop terms are added with one
identity matmul from the core's own table block.  dinv scaling is folded
into the tables (dinv[src], on write) and into the relu's per-partition
scalar multiply (dinv[dst], on PSUM readout).

Layer 1 messages (x[src]) are materialized host-side in SBUF layout and
streamed sequentially — no device-side gather at all.  Layer 2 messages
(h1[src]) only exist on device: each core writes its h1 slice, an AllGather
publishes the full table, and batched dma_gather pulls the per-edge rows
(the int16 index limit is dodged by splitting chunks into two streams
gathering from two base-offset views of the table).
"""

import os
import sys

for _p in ("/opt/trn_rl_repo", "/root/.axon_site/_ro/trn_rl_repo"):
    if os.path.isdir(_p) and _p not in sys.path:
        sys.path.append(_p)

import numpy as np
import ml_dtypes

import concourse.bacc as bacc
import concourse.tile as tile
from concourse import mybir
from concourse.bass_utils import run_bass_kernel_spmd

NC = 8          # cores
BLK = 128       # dst rows per aggregation block
CHUNK = 128     # edges per matmul chunk (PE contraction dim)
G = 32          # chunks per dma_gather call (layer 2)
GB = 16         # chunks per msg/S stream DMA
SPLIT = 32768   # int16 index limit for dma_gather


def _pack_idx(flat: np.ndarray) -> np.ndarray:
    """dma_gather index layout: [128, n/16] int16, idx i at [i%16, i//16],
    replicated across the 8 gpsimd cores (partition groups of 16)."""
    n = flat.shape[0]
    assert n % 16 == 0
    return np.ascontiguousarray(np.tile(flat.reshape(n // 16, 16).T, (8, 1)))


class _Plan:
    """Host-side schedule + per-core streams (shared by both layers)."""

    def __init__(self, n_nodes: int, edge_index: np.ndarray):
        assert n_nodes % NC == 0
        self.N = n_nodes
        self.ROWS = n_nodes // NC
        self.NBLK = (self.ROWS + BLK - 1) // BLK
        self.last_rows = self.ROWS - (self.NBLK - 1) * BLK

        src = np.asarray(edge_index[0], dtype=np.int64)
        dst = np.asarray(edge_index[1], dtype=np.int64)

        # self-loops are NOT materialized as edge slots: each block adds its
        # own table rows via an identity matmul.  deg still counts them.
        deg = (np.bincount(dst, minlength=n_nodes) + 1).astype(np.float64)
        self.dinv = (1.0 / np.sqrt(deg)).astype(np.float32)

        core = dst // self.ROWS
        rem = dst % self.ROWS
        blk = rem // BLK
        drel = rem % BLK
        hi = (src >= SPLIT).astype(np.int64)

        key = (core * self.NBLK + blk) * 2 + hi
        counts = np.bincount(key, minlength=NC * self.NBLK * 2).reshape(
            NC, self.NBLK, 2
        )
        K = -(-counts // CHUNK)
        K = K.max(axis=0)  # [NBLK, 2] chunk counts, shared by all cores
        self.Klo = K[:, 0].astype(np.int64)
        self.Khi = K[:, 1].astype(np.int64)
        self.C_lo = int(self.Klo.sum())
        self.C_hi = int(self.Khi.sum())
        self.C = self.C_lo + self.C_hi

        base = lambda k: np.concatenate([[0], np.cumsum(k)])
        self.base_lo = base(self.Klo)
        self.base_hi = base(self.Khi)
        self.base_g = base(self.Klo + self.Khi)

        # per-core streams
        self.S = []        # [128, C*128] bf16 selection columns
        self.slotsrc = []  # [C*128] int64 source node per slot (pad: 0/SPLIT)
        self.idxlo = []    # [128, C_lo*8] int16
        self.idxhi = []    # [128, C_hi*8] int16
        self.dinv_col = []   # [128, NBLK] f32
        self.dinv2_col = []  # [128, NBLK] f32
        for c in range(NC):
            m = core == c
            sa, bb, dd, ss = src[m], blk[m], drel[m], hi[m]
            k2 = bb * 2 + ss
            order = np.argsort(k2, kind="stable")
            sa, bb, dd, ss, k2 = (
                sa[order], bb[order], dd[order], ss[order], k2[order],
            )
            change = np.r_[True, k2[1:] != k2[:-1]]
            startidx = np.flatnonzero(change)
            sizes = np.diff(np.r_[startidx, len(k2)])
            pos = np.arange(len(k2)) - np.repeat(startidx, sizes)
            ch = pos // CHUNK
            lane = pos % CHUNK
            q = self.base_g[bb] + np.where(ss == 0, ch, self.Klo[bb] + ch)
            pstream = np.where(ss == 0, self.base_lo[bb], self.base_hi[bb]) + ch

            S = np.zeros((128, self.C, 128), dtype=ml_dtypes.bfloat16)
            S[lane, q, dd] = 1.0
            self.S.append(np.ascontiguousarray(S.reshape(128, self.C * 128)))

            # slot -> source node (in global chunk order); pads read row 0 of
            # the stream's table view so indices stay in range.
            slotsrc = np.zeros(self.C * CHUNK, np.int64)
            hi_slots = np.zeros(self.C * CHUNK, bool)
            kk = np.repeat(self.Klo, 1)
            for b in range(self.NBLK):
                s0 = (self.base_g[b] + self.Klo[b]) * CHUNK
                s1 = self.base_g[b + 1] * CHUNK
                hi_slots[s0:s1] = True
            slotsrc[hi_slots] = SPLIT
            slot = q * CHUNK + lane
            slotsrc[slot] = sa
            self.slotsrc.append(slotsrc)

            # L2 gather index streams (lo chunks then hi chunks, per stream)
            idx_lo = np.zeros(max(self.C_lo, 1) * CHUNK, np.int16)
            idx_hi = np.zeros(max(self.C_hi, 1) * CHUNK, np.int16)
            lo_m = ss == 0
            idx_lo[pstream[lo_m] * CHUNK + lane[lo_m]] = sa[lo_m].astype(
                np.int16
            )
            if self.C_hi:
                idx_hi[pstream[~lo_m] * CHUNK + lane[~lo_m]] = (
                    sa[~lo_m] - SPLIT
                ).astype(np.int16)
            self.idxlo.append(_pack_idx(idx_lo))
            self.idxhi.append(_pack_idx(idx_hi))

            dv = np.ones(self.NBLK * BLK, np.float32)
            dv[: self.ROWS] = self.dinv[c * self.ROWS : (c + 1) * self.ROWS]
            dv = dv.reshape(self.NBLK, BLK).T.copy()  # [128, NBLK]
            self.dinv_col.append(dv)
            self.dinv2_col.append(dv * dv)

    def msg1(self, xg_full: np.ndarray, c: int) -> np.ndarray:
        """Layer-1 message stream for core c, already in SBUF layout
        [128 slot-lane partitions, C*128 (chunk-major, feature-minor)]."""
        m = xg_full[self.slotsrc[c]]  # [C*128, F]
        F = m.shape[1]
        return np.ascontiguousarray(
            m.reshape(self.C, CHUNK, F).transpose(1, 0, 2).reshape(
                128, self.C * F
            )
        )

    def signature(self, f_in, f_out, has_b1, has_b2):
        return (
            self.N, f_in, f_out, has_b1, has_b2,
            tuple(self.Klo), tuple(self.Khi),
        )


def _build(plan: _Plan, f_in: int, f_out: int, has_b1: bool, has_b2: bool):
    """Build + compile the SPMD Bass program (one NEFF, runs on all 8 cores)."""
    N, ROWS, NBLK = plan.N, plan.ROWS, plan.NBLK
    C, C_lo, C_hi = plan.C, plan.C_lo, plan.C_hi
    bf16, f32, i16 = mybir.dt.bfloat16, mybir.dt.float32, mybir.dt.int16

    nc = bacc.Bacc("TRN2", target_bir_lowering=False, debug=False,
                   enable_asserts=True, num_devices=NC)

    xgl = nc.dram_tensor("xgl", [ROWS, f_in], bf16, kind="ExternalInput")
    msg1 = nc.dram_tensor("msg1", [128, C * f_in], bf16, kind="ExternalInput")
    smat = nc.dram_tensor("smat", [128, C * 128], bf16, kind="ExternalInput")
    ident_in = nc.dram_tensor("ident", [128, 128], bf16, kind="ExternalInput")
    w1 = nc.dram_tensor("w1", [f_in, f_in], f32, kind="ExternalInput")
    w2 = nc.dram_tensor("w2", [f_in, f_out], f32, kind="ExternalInput")
    if has_b1:
        b1bc = nc.dram_tensor("b1bc", [128, f_in], f32, kind="ExternalInput")
    if has_b2:
        b2bc = nc.dram_tensor("b2bc", [128, f_out], f32, kind="ExternalInput")
    idxlo = nc.dram_tensor("idxlo", [128, max(C_lo, 1) * 8], i16,
                           kind="ExternalInput")
    if C_hi:
        idxhi = nc.dram_tensor("idxhi", [128, C_hi * 8], i16,
                               kind="ExternalInput")
    dinv_c = nc.dram_tensor("dinv_c", [128, NBLK], f32, kind="ExternalInput")
    dinv2_c = nc.dram_tensor("dinv2_c", [128, NBLK], f32, kind="ExternalInput")
    out_ext = nc.dram_tensor("out", [ROWS, f_out], f32, kind="ExternalOutput")

    with tile.TileContext(nc) as tc:
        with (
            tc.tile_pool(name="meta", bufs=1) as pm,
            tc.tile_pool(name="work", bufs=2) as pw,
            tc.tile_pool(name="psum", space="PSUM", bufs=2) as pp,
            tc.tile_pool(name="dram", space="DRAM", bufs=1) as pd,
        ):
            # ---- persistent metadata in SBUF ----
            ident_t = pm.tile([128, 128], bf16)
            nc.sync.dma_start(ident_t[:], ident_in[:])
            idxlo_t = pm.tile([128, max(C_lo, 1) * 8], i16)
            nc.sync.dma_start(idxlo_t[:], idxlo[:])
            if C_hi:
                idxhi_t = pm.tile([128, C_hi * 8], i16)
                nc.sync.dma_start(idxhi_t[:], idxhi[:])
            dinv_t = pm.tile([128, NBLK], f32)
            nc.sync.dma_start(dinv_t[:], dinv_c[:])
            dinv2_t = pm.tile([128, NBLK], f32)
            nc.sync.dma_start(dinv2_t[:], dinv2_c[:])

            w1f = pm.tile([f_in, f_in], f32)
            nc.sync.dma_start(w1f[:], w1[:])
            w1_t = pm.tile([f_in, f_in], bf16)
            nc.vector.tensor_copy(w1_t[:], w1f[:])
            w2f = pm.tile([f_in, f_out], f32)
            nc.sync.dma_start(w2f[:], w2[:])
            w2_t = pm.tile([f_in, f_out], bf16)
            nc.vector.tensor_copy(w2_t[:], w2f[:])
            if has_b1:
                b1_t = pm.tile([128, f_in], f32)
                nc.sync.dma_start(b1_t[:], b1bc[:])
            if has_b2:
                b2_t = pm.tile([128, f_out], f32)
                nc.sync.dma_start(b2_t[:], b2bc[:])

            # The DMAGatherAnt instruction struct only has room for ONE sync
            # wait command (walrus setupSyncWait limit).  Absorb the idx-load
            # dependencies into a throwaway gpsimd DMA so every dma_gather
            # needs at most one wait (collective done / msg-buf WAR).
            scratch = pm.tile([1, 16], i16, name="scratch")
            nc.gpsimd.dma_start(scratch[0:1, 0:16], idxlo_t[0:1, 0:16])
            if C_hi:
                nc.gpsimd.dma_start(scratch[0:1, 0:16], idxhi_t[0:1, 0:16])

            ag2_in = pd.tile([ROWS, f_in], bf16)
            h1_tab = pd.tile([N, f_in], bf16, addr_space="Shared")

            def stream_tile(dram_src, q0, n, tag):
                """[128, n*128] slice of a host-precomputed [128, C*128]
                stream, loaded contiguously at line rate."""
                t = pw.tile([128, GB * 128], bf16, tag=tag, bufs=4,
                            name=f"{tag}_{q0}")
                nc.sync.dma_start(
                    t[:, : n * 128], dram_src[:, q0 * 128 : (q0 + n) * 128]
                )
                return t

            def emit_layer(layer, w_t, fo, bias_t, scale_t, local_tab):
                # msg source: layer 0 streams msg1; layer 1 gathers h1_tab
                msg_tiles = {}
                s_tiles = {}

                def get_msg(q):
                    if layer == 0:
                        j, col = divmod(q, GB)
                        if j not in msg_tiles:
                            n = min(GB, C - j * GB)
                            msg_tiles[j] = stream_tile(msg1, j * GB, n, "msg")
                        return msg_tiles[j], col
                    # layer 1: chunk q -> (stream, pos)
                    b = int(np.searchsorted(plan.base_g, q, side="right")) - 1
                    i = q - int(plan.base_g[b])
                    if i < int(plan.Klo[b]):
                        s, p = 0, int(plan.base_lo[b]) + i
                    else:
                        s, p = 1, int(plan.base_hi[b]) + i - int(plan.Klo[b])
                    j, col = divmod(p, G)
                    if (s, j) not in msg_tiles:
                        cs = C_lo if s == 0 else C_hi
                        n = min(G, cs - j * G)
                        mt = pw.tile([128, G * CHUNK], bf16, tag="gat",
                                     bufs=12, name=f"gat_{s}_{j}")
                        view = (h1_tab[0 : min(SPLIT, N), :] if s == 0
                                else h1_tab[SPLIT:N, :])
                        it = idxlo_t if s == 0 else idxhi_t
                        nc.gpsimd.dma_gather(
                            mt[:, : n * CHUNK].rearrange(
                                "p (g e) -> p g e", e=f_in),
                            view,
                            it[:, j * G * 8 : (j * G + n) * 8],
                            n * CHUNK, n * CHUNK, f_in,
                            single_packet=False,
                        )
                        msg_tiles[(s, j)] = mt
                    return msg_tiles[(s, j)], col

                def get_s(q):
                    j, col = divmod(q, GB)
                    if (layer, j) not in s_tiles:
                        n = min(GB, C - j * GB)
                        s_tiles[(layer, j)] = stream_tile(
                            smat, j * GB, n, "sel")
                    return s_tiles[(layer, j)], col

                q = 0
                for b in range(NBLK):
                    rows = plan.last_rows if b == NBLK - 1 else BLK
                    nch = int(plan.Klo[b]) + int(plan.Khi[b])
                    psum = pp.tile([128, 128], f32, tag="agg", bufs=4,
                                   name=f"agg_l{layer}_{b}")
                    # self-loop contribution: psum[f,d] += local[dd,f]*I[dd,d]
                    loc = pw.tile([128, 128], bf16, tag="loc", bufs=4,
                                  name=f"loc_l{layer}_{b}")
                    nc.sync.dma_start(
                        loc[:rows, :],
                        local_tab[b * BLK : b * BLK + rows, :],
                    )
                    # rows < BLK leaves stale data in loc[rows:]; identity
                    # routing sends row dd only to psum column dd, and
                    # columns >= rows are never read back, so it's harmless.
                    nc.tensor.matmul(psum[:], lhsT=loc[:], rhs=ident_t[:],
                                     start=True, stop=(nch == 0))
                    for i in range(nch):
                        mt, mcol = get_msg(q)
                        st, scol = get_s(q)
                        nc.tensor.matmul(
                            psum[:],
                            lhsT=mt[:, mcol * CHUNK : (mcol + 1) * CHUNK],
                            rhs=st[:, scol * 128 : (scol + 1) * 128],
                            start=False, stop=(i == nch - 1),
                        )
                        q += 1
                    aggT = pw.tile([128, 128], bf16, tag="aggT", bufs=4,
                                   name=f"aggT_l{layer}_{b}")
                    nc.vector.tensor_copy(aggT[:], psum[:])
                    ph = pp.tile([128, fo], f32, tag="hout", bufs=2,
                                 name=f"ph_l{layer}_{b}")
                    nc.tensor.matmul(ph[:], lhsT=aggT[:], rhs=w_t[:],
                                     start=True, stop=True)
                    hsb = pw.tile([128, fo], bf16 if layer == 0 else f32,
                                  tag=f"hsb{layer}", bufs=4,
                                  name=f"hsb_l{layer}_{b}")
                    if bias_t is None:
                        # relu(dinv*x)*k == max(x,0)*(dinv*k): one DVE op
                        nc.vector.tensor_scalar(
                            out=hsb[:], in0=ph[:],
                            scalar1=0.0,
                            scalar2=scale_t[:, b : b + 1],
                            op0=mybir.AluOpType.max,
                            op1=mybir.AluOpType.mult,
                        )
                    else:
                        tmp = pw.tile([128, fo], f32, tag=f"tmp{layer}",
                                      bufs=2, name=f"tmp_l{layer}_{b}")
                        nc.vector.scalar_tensor_tensor(
                            out=tmp[:], in0=ph[:],
                            scalar=dinv_t[:, b : b + 1], in1=bias_t[:],
                            op0=mybir.AluOpType.mult,
                            op1=mybir.AluOpType.add,
                        )
                        if layer == 0:
                            nc.vector.tensor_scalar(
                                out=hsb[:], in0=tmp[:], scalar1=0.0,
                                scalar2=dinv_t[:, b : b + 1],
                                op0=mybir.AluOpType.max,
                                op1=mybir.AluOpType.mult,
                            )
                        else:
                            nc.vector.tensor_scalar(
                                out=hsb[:], in0=tmp[:], scalar1=0.0,
                                scalar2=None,
                                op0=mybir.AluOpType.max,
                                op1=mybir.AluOpType.bypass,
                            )
                    dst_rows = slice(b * BLK, b * BLK + rows)
                    if layer == 0:
                        nc.sync.dma_start(ag2_in[dst_rows, :], hsb[:rows, :])
                    else:
                        nc.sync.dma_start(out_ext[dst_rows, :], hsb[:rows, :])

            # layer 1: table rows pre-scaled by dinv[src] on host; output row
            # d is relu(dinv_d * (agg @ W1) + b1) * dinv_d (layer-2 table)
            emit_layer(0, w1_t, f_in, b1_t if has_b1 else None,
                       dinv2_t, xgl)
            nc.gpsimd.collective_compute(
                "AllGather", mybir.AluOpType.bypass,
                replica_groups=[list(range(NC))],
                ins=[ag2_in[:]], outs=[h1_tab[:]],
            )
            # layer 2: out row d = relu(dinv_d * (agg @ W2) + b2)
            emit_layer(1, w2_t, f_out, b2_t if has_b2 else None,
                       dinv_t, ag2_in)

    nc.compile()
    return nc


_cache: dict = {}


def _get_program(plan, f_in, f_out, has_b1, has_b2):
    key = plan.signature(f_in, f_out, has_b1, has_b2)
    if key not in _cache:
        _cache[key] = _build(plan, f_in, f_out, has_b1, has_b2)
    return _cache[key]


def _run(x, edge_index, W1, b1, W2, b2, trace=False, trace_cores=None):
    x = np.asarray(x, dtype=np.float32)
    W1 = np.asarray(W1, dtype=np.float32)
    W2 = np.asarray(W2, dtype=np.float32)
    b1 = np.asarray(b1, dtype=np.float32)
    b2 = np.asarray(b2, dtype=np.float32)
    N, f_in = x.shape
    f_out = W2.shape[1]
    has_b1 = bool(np.any(b1))
    has_b2 = bool(np.any(b2))

    plan = _Plan(N, np.asarray(edge_index))
    nc = _get_program(plan, f_in, f_out, has_b1, has_b2)

    in_maps = _make_in_maps(plan, x, W1, b1, W2, b2)

    kw = {}
    if trace:
        kw = dict(trace=True)
        if trace_cores is not None:
            kw["trace_cores"] = trace_cores
    res = run_bass_kernel_spmd(nc, in_maps, core_ids=list(range(NC)), **kw)
    out = np.concatenate([res.results[c]["out"] for c in range(NC)], axis=0)
    return out, res


def _make_in_maps(plan, x, W1, b1, W2, b2):
    has_b1 = bool(np.any(b1))
    has_b2 = bool(np.any(b2))
    xg_full = (x * plan.dinv[:, None]).astype(ml_dtypes.bfloat16)
    ident = np.eye(128, dtype=ml_dtypes.bfloat16)
    in_maps = []
    for c in range(NC):
        m = dict(
            xgl=np.ascontiguousarray(
                xg_full[c * plan.ROWS : (c + 1) * plan.ROWS]
            ),
            msg1=plan.msg1(xg_full, c),
            smat=plan.S[c],
            ident=ident,
            w1=W1, w2=W2,
            idxlo=plan.idxlo[c],
            dinv_c=plan.dinv_col[c],
            dinv2_c=plan.dinv2_col[c],
        )
        if plan.C_hi:
            m["idxhi"] = plan.idxhi[c]
        if has_b1:
            m["b1bc"] = np.ascontiguousarray(np.tile(b1, (128, 1)))
        if has_b2:
            m["b2bc"] = np.ascontiguousarray(np.tile(b2, (128, 1)))
        in_maps.append(m)
    return in_maps


def kernel(x, edge_index, W1, b1, W2, b2):
    out, _ = _run(x, edge_index, W1, b1, W2, b2)
    return out
